# revision 18
# baseline (speedup 1.0000x reference)
"""Distributed Trainium2 kernel for causal multi-head attention with RoPE.

Problem: B=2, S=2048, E=2048, N=16 heads, H=128 head-dim.
Sharding: 8 cores = 2 (batch, data axis) x 4 (head groups, model axis).
Each core:
  phase 1: Q/K/V projections for its 4 heads (bf16 matmuls, f32 accum),
           RoPE applied to q^T/k^T in [H, S] layout.  Q runs first over
           all four 512-col s-chunks (sc pairs share one W stationary
           load), then K+V per sc pair, so q^T frees early for the next
           loop iteration and LDWEIGHTS count is halved for Q/K.
  phase 2: causal attention, transposed-score formulation, sc-major:
           for each s-chunk, all 4 heads.  Scores L^T[t,s] (lhsT = k^T
           block, rhs = q^T chunk), exp PSUM->SBUF bf16.  Softmax
           denominators: full 512-wide t-blocks are quad-reduced on the
           DVE (bf16 tree adds) so the ones-column matmul streams 1/4
           of the columns; diagonal blocks feed the matmul directly.
           Normalization (reciprocal -> gpsimd partition_broadcast ->
           DVE multiply) is deferred by one head so the PE never waits
           on it.
  phase 3: two AllGathers per 4-core group split by s-columns (0:1536
           after sc 0-2 finish, 1536:2048 at the end); a^T for all 16
           heads is re-read s-chunk-major so the output projection can
           start on the first s-tile as soon as its chunk lands.  Each
           core computes o for a different 512-wide E-slice.
Host side re-assembles the 8 [2048, 512] E-chunks into [2, 2048, 2048].

DMA queues: phase-1/3 loads on sync (SP), output writes on the scalar
(Activation) queue, a^T stores + collectives on gpsimd (Pool) — so the
sync queue reaches the next iteration's weight loads while the current
iteration is still in attention.
"""

import contextlib

import numpy as np
import ml_dtypes

import concourse.mybir as mybir
import concourse.tile as tile
from concourse import bacc
from concourse.bass_utils import run_bass_kernel_spmd

B, S, E, N, H = 2, 2048, 2048, 16, 128
P = 128
NCORES = 8
NH_LOC = N // 4          # 4 heads per core
ECHUNK = E // 4          # 512 output-embedding columns per core
EC = E // P              # 16 contraction chunks
ST = S // P              # 16 seq tiles of 128
F32 = mybir.dt.float32
BF16 = mybir.dt.bfloat16

REPLICA_GROUPS = [[0, 1, 2, 3], [4, 5, 6, 7]]

TRACE = False
LAST_RESULTS = None
BC_VIA_GPSIMD = True     # reciprocal broadcast on Pool instead of PE matmul


def _rope_tables():
    """cos^T / sin^T tables [H, S] f32, sign-folded and scaled by 128**-0.25.

    Matches reference.sine_table computed in f32 (angles formed with f32
    arithmetic, sin/cos evaluated in f64 of the f32 angle).
    """
    fraction = np.arange(0, H, 2, dtype=np.float32) / np.float32(H)
    timescale = np.float32(10000.0) ** fraction
    inv = (np.float32(1.0) / timescale).astype(np.float32)
    ang = (np.arange(S, dtype=np.float32)[:, None] * inv[None, :]).astype(np.float32)
    ang = np.concatenate([ang, ang], axis=1)        # [S, H]
    sin = np.sin(ang.astype(np.float64))
    cos = np.cos(ang.astype(np.float64))
    scale = float(H) ** -0.25
    cosT = (cos.T * scale).astype(np.float32)        # [H, S]
    sinT = (sin.T * scale).astype(np.float32)
    sinT[:H // 2] *= -1.0                            # sign of rotate_half folded in
    return np.ascontiguousarray(cosT), np.ascontiguousarray(sinT)


def _phase1(nc, tc, qT_sc, kT_sc, v_sc, cos_sb, sin_sb, cos_srcs,
            xqT_r, xkT_r, wq_r, wk_r, wv_r, sfx=""):
    """QKV proj + RoPE -> per-s-chunk qT/kT [h, n, 512] and v [t, tt, nh].

    Pass 1: Q for all four s-chunks (paired, shared W stationary).
    Pass 2: K + V per s-chunk pair (sharing the xk strips).
    """
    with (
        tc.tile_pool(name="wqkv" + sfx, bufs=1) as w_pool,
        tc.tile_pool(name="xin" + sfx, bufs=2) as x_pool,
        tc.tile_pool(name="rope_tmp" + sfx, bufs=3) as tmp_pool,
        tc.tile_pool(name="ph1_psum" + sfx, bufs=2, space="PSUM") as pp1,
    ):
        wq_sb = w_pool.tile([P, EC, NH_LOC * H], BF16)
        nc.sync.dma_start(wq_sb[:, 0:4, :], wq_r[:, 0:4, :])

        def load_strips(xT_r, scs):
            # xq and xk share the tag space: by the time the K+V pass
            # needs strips, the Q pass has consumed the same-tag buffer.
            st = {}
            for si, sc in enumerate(scs):
                cols = slice(sc * 512, (sc + 1) * 512)
                for j in range(4):
                    t = x_pool.tile([P, 4, 512], BF16, tag=f"x{si}{j}")
                    nc.sync.dma_start(t[:], xT_r[:, 4 * j:4 * j + 4, cols])
                    st[(sc, j)] = t
            return st

        xq_st = load_strips(xqT_r, (0, 1))
        # deferred loads, queued behind what pass 1 needs first
        nc.sync.dma_start(cos_sb[:], cos_srcs[0])
        nc.sync.dma_start(sin_sb[:], cos_srcs[1])
        for j in range(1, 4):
            nc.sync.dma_start(wq_sb[:, 4 * j:4 * j + 4, :],
                              wq_r[:, 4 * j:4 * j + 4, :])
        wk_sb = w_pool.tile([P, EC, NH_LOC * H], BF16)
        nc.sync.dma_start(wk_sb[:], wk_r)
        wv_sb = w_pool.tile([P, EC, NH_LOC * H], BF16)

        def rope(ps, dst, cols):
            """dst = ps*cos + shift128(ps)*sin  (bf16 out), cols into S."""
            w = cols.stop - cols.start
            t_sin = tmp_pool.tile([P, 512], F32, tag="t_sin")
            nc.vector.tensor_tensor(
                out=t_sin[0:64, :w], in0=ps[64:128, :w],
                in1=sin_sb[0:64, cols], op=mybir.AluOpType.mult)
            nc.vector.tensor_tensor(
                out=t_sin[64:128, :w], in0=ps[0:64, :w],
                in1=sin_sb[64:128, cols], op=mybir.AluOpType.mult)
            t_cos = tmp_pool.tile([P, 512], F32, tag="t_cos")
            nc.vector.tensor_tensor(
                out=t_cos[:, :w], in0=ps[:, :w],
                in1=cos_sb[:, cols], op=mybir.AluOpType.mult)
            nc.vector.tensor_add(out=dst, in0=t_cos[:, :w], in1=t_sin[:, :w])

        # ---------------- pass 1: Q over sc pairs ----------------
        for half in range(2):
            scs = (2 * half, 2 * half + 1)
            if half == 1:
                xq_st.update(load_strips(xqT_r, scs))
            for n in range(NH_LOC):
                hs = slice(n * H, (n + 1) * H)
                ps = [pp1.tile([P, 512], F32, tag=f"pp{i}", name=f"psq{i}")
                      for i in range(2)]
                for ec in range(EC):
                    for i, sc in enumerate(scs):
                        nc.tensor.matmul(
                            ps[i], wq_sb[:, ec, hs],
                            xq_st[(sc, ec // 4)][:, ec % 4, :],
                            start=(ec == 0), stop=(ec == EC - 1),
                            skip_group_check=True)
                for i, sc in enumerate(scs):
                    rope(ps[i], qT_sc[sc][:, n, :],
                         slice(sc * 512, (sc + 1) * 512))

        # ---------------- pass 2: K + V over sc pairs ----------------
        xk_st = load_strips(xkT_r, (0, 1))
        nc.sync.dma_start(wv_sb[:], wv_r)
        for half in range(2):
            scs = (2 * half, 2 * half + 1)
            if half == 1:
                xk_st.update(load_strips(xkT_r, scs))
            for n in range(NH_LOC):
                hs = slice(n * H, (n + 1) * H)
                ps = [pp1.tile([P, 512], F32, tag=f"pp{i}", name=f"psk{i}")
                      for i in range(2)]
                for ec in range(EC):
                    for i, sc in enumerate(scs):
                        nc.tensor.matmul(
                            ps[i], wk_sb[:, ec, hs],
                            xk_st[(sc, ec // 4)][:, ec % 4, :],
                            start=(ec == 0), stop=(ec == EC - 1),
                            skip_group_check=True)
                for i, sc in enumerate(scs):
                    rope(ps[i], kT_sc[sc][:, n, :],
                         slice(sc * 512, (sc + 1) * 512))
            for sc in scs:
                for tt in range(4):                 # v for 4 t-tiles
                    tsl = slice(tt * P, (tt + 1) * P)
                    psv = pp1.tile([P, 512], F32, tag="psv")
                    for ec in range(EC):
                        nc.tensor.matmul(
                            psv, xk_st[(sc, ec // 4)][:, ec % 4, tsl],
                            wv_sb[:, ec, :],
                            start=(ec == 0), stop=(ec == EC - 1))
                    nc.scalar.copy(out=v_sc[sc][:, tt, :], in_=psv[:])


def _attn_sc(nc, tc, pools, sc, heads, qT_sc, kT_sc, v_sc, consts, ag_in,
             pending):
    """Causal attention chains for s-chunk sc, given heads.

    Normalization of each (sc, head) chain is deferred until after the
    next chain's matmuls are issued (pending holds one entry).
    """
    (elt_pool, qsum_pool, rc_pool, bc_pool, at_pool,
     lt_psum, av_psum, sm_psum) = pools
    maskT, ones_col = consts

    for n in heads:
        hs = slice(n * H, (n + 1) * H)
        avp = av_psum.tile([P, 512], F32, tag="av")

        # full t-blocks in quads: pv accumulates per block; the block
        # sums are quad-reduced on DVE (bf16 tree) into running ts.
        ts = None
        for q in range(sc):
            elts = []
            for j in range(4):
                tb = 4 * q + j
                kblk = kT_sc[q][:, n, j * P:(j + 1) * P]
                lt = lt_psum.tile([P, 512], F32, tag="lt")
                nc.tensor.matmul(lt[:], kblk, qT_sc[sc][:, n, :],
                                 start=True, stop=True)
                elt = elt_pool.tile([P, 512], BF16, tag=f"elt{j}")
                nc.scalar.activation(
                    out=elt[:], in_=lt[:],
                    func=mybir.ActivationFunctionType.Exp)
                nc.tensor.matmul(
                    avp[:], v_sc[q][:, j, hs], elt[:],
                    start=(tb == 0), stop=False, skip_group_check=True)
                elts.append(elt)
            s01 = qsum_pool.tile([P, 512], BF16, tag="s01")
            nc.vector.tensor_add(out=s01[:], in0=elts[0][:], in1=elts[1][:])
            s23 = qsum_pool.tile([P, 512], BF16, tag="s23")
            nc.vector.tensor_add(out=s23[:], in0=elts[2][:], in1=elts[3][:])
            sq = qsum_pool.tile([P, 512], BF16, tag=f"sq{q % 2}")
            nc.vector.tensor_add(out=sq[:], in0=s01[:], in1=s23[:])
            if ts is not None:
                nc.vector.tensor_add(out=sq[:], in0=sq[:], in1=ts[:])
            ts = sq

        # diagonal 4 blocks (w = 512, 384, 256, 128): their column sums
        # are accumulated on the DVE into ds so smp gets a single
        # 512-column stream per chain
        ds = qsum_pool.tile([P, 512], BF16, tag="ds")
        for j in range(4):
            tb = 4 * sc + j
            w = 512 - j * P
            col0 = 512 - w
            kblk = kT_sc[sc][:, n, j * P:(j + 1) * P]
            lt = lt_psum.tile([P, 512], F32, tag="lt")
            nc.tensor.matmul(
                lt[:, :w], kblk, qT_sc[sc][:, n, col0:512],
                start=True, stop=True)
            elt = elt_pool.tile([P, 512], BF16, tag=f"elt{j}")
            nc.scalar.activation(
                out=elt[:, :w], in_=lt[:, :w],
                func=mybir.ActivationFunctionType.Exp)
            nc.vector.tensor_mul(
                out=elt[:, 0:P], in0=elt[:, 0:P], in1=maskT[:])
            nc.tensor.matmul(
                avp[:, col0:512], v_sc[sc][:, j, hs], elt[:, :w],
                start=(tb == 0), stop=(j == 3), skip_group_check=True)
            if j == 0:
                nc.vector.tensor_copy(out=ds[:], in_=elt[:])
            else:
                nc.vector.tensor_add(
                    out=ds[:, col0:512], in0=ds[:, col0:512],
                    in1=elt[:, :w])
        if ts is not None:
            nc.vector.tensor_add(out=ds[:], in0=ds[:], in1=ts[:])
        smp = sm_psum.tile([1, 512], F32, tag="sm")
        nc.tensor.matmul(smp[:], ones_col[:], ds[:],
                         start=True, stop=True)

        if pending:
            _flush_one(nc, pools, ag_in, pending)
        pending.append((n, sc, avp, smp))


def _build(reps=1, with_cc=True, loop_trips=0):
    nc = bacc.Bacc("TRN2", target_bir_lowering=False, debug=False,
                   num_devices=NCORES)

    xqT = nc.dram_tensor("xqT", [E, S], BF16, kind="ExternalInput")
    xkT = nc.dram_tensor("xkT", [E, S], BF16, kind="ExternalInput")
    wq = nc.dram_tensor("wq", [E, NH_LOC * H], BF16, kind="ExternalInput")
    wk = nc.dram_tensor("wk", [E, NH_LOC * H], BF16, kind="ExternalInput")
    wv = nc.dram_tensor("wv", [E, NH_LOC * H], BF16, kind="ExternalInput")
    wo = nc.dram_tensor("wo", [N * H, ECHUNK], BF16, kind="ExternalInput")
    cosT = nc.dram_tensor("cosT", [H, S], F32, kind="ExternalInput")
    sinT = nc.dram_tensor("sinT", [H, S], F32, kind="ExternalInput")
    out = nc.dram_tensor("out", [S, ECHUNK], F32, kind="ExternalOutput")

    # s-chunk-major so each AllGather input slice is contiguous
    ag_in = nc.dram_tensor("ag_in", [4, NH_LOC * H, 512], BF16)
    ag_out1 = nc.dram_tensor("ag_out1", [4, 3, NH_LOC * H, 512], BF16)
    ag_out2 = nc.dram_tensor("ag_out2", [4, NH_LOC * H, 512], BF16)

    xqT_r = xqT.ap().rearrange("(eo p) s -> p eo s", p=P)    # [128, 16, 2048]
    xkT_r = xkT.ap().rearrange("(eo p) s -> p eo s", p=P)
    wq_r = wq.ap().rearrange("(eo p) m -> p eo m", p=P)      # [128, 16, 512]
    wk_r = wk.ap().rearrange("(eo p) m -> p eo m", p=P)
    wv_r = wv.ap().rearrange("(eo p) m -> p eo m", p=P)
    wo_r = wo.ap().rearrange("(n p) e -> p n e", p=P)        # [128, 16, 512]

    with tile.TileContext(nc) as tc:
        with tc.tile_pool(name="const", bufs=1) as const_pool:
            # multiplicative causal mask for L^T diagonal blocks:
            # keep where s - t >= 0
            maskT = const_pool.tile([P, P], BF16)
            nc.gpsimd.memset(maskT, 1.0)
            nc.gpsimd.affine_select(
                out=maskT, in_=maskT,
                compare_op=mybir.AluOpType.is_ge, fill=0.0,
                base=0, pattern=[[1, P]], channel_multiplier=-1)
            ones_col = const_pool.tile([P, 1], BF16)
            nc.gpsimd.memset(ones_col, 1.0)
            cos_sb = const_pool.tile([P, S], F32)
            sin_sb = const_pool.tile([P, S], F32)
            cos_srcs = (cosT.ap(), sinT.ap())

            loop_cm = tc.For_i(0, loop_trips, 1) if loop_trips else \
                contextlib.nullcontext()
            with loop_cm:
                _build_body(nc, tc, reps, with_cc, out, ag_in,
                            ag_out1, ag_out2, cos_sb, sin_sb,
                            cos_srcs, maskT, ones_col,
                            xqT_r, xkT_r, wq_r, wk_r, wv_r, wo_r)

    nc.compile()
    return nc


def _build_body(nc, tc, reps, with_cc, out, ag_in, ag_out1, ag_out2,
                cos_sb, sin_sb, cos_srcs, maskT, ones_col, xqT_r, xkT_r,
                wq_r, wk_r, wv_r, wo_r):
    for rep in range(reps):
        sfx = f"_r{rep}" if reps > 1 else ""
        with tc.tile_pool(name="qkv" + sfx, bufs=1) as qkv_pool:
            qT_sc = [qkv_pool.tile([P, NH_LOC, 512], BF16, name=f"qT{sc}")
                     for sc in range(4)]
            kT_sc = [qkv_pool.tile([P, NH_LOC, 512], BF16, name=f"kT{sc}")
                     for sc in range(4)]
            v_sc = [qkv_pool.tile([P, 4, NH_LOC * H], BF16, name=f"v{sc}")
                    for sc in range(4)]

            _phase1(nc, tc, qT_sc, kT_sc, v_sc, cos_sb, sin_sb, cos_srcs,
                    xqT_r, xkT_r, wq_r, wk_r, wv_r, sfx)

            # prefetch WO for phase 3 (SBUF freed by phase-1 pools)
            with tc.tile_pool(name="wo_pool" + sfx, bufs=1) as wo_pool:
                wo_sb = wo_pool.tile([P, N, ECHUNK], BF16)
                nc.sync.dma_start(wo_sb[:], wo_r)

                with (
                    tc.tile_pool(name="elt" + sfx, bufs=2) as elt_pool,
                    tc.tile_pool(name="qsum" + sfx, bufs=2) as qsum_pool,
                    tc.tile_pool(name="rc" + sfx, bufs=2) as rc_pool,
                    tc.tile_pool(name="bc" + sfx, bufs=2) as bc_pool,
                    tc.tile_pool(name="at" + sfx, bufs=2) as at_pool,
                    # ao lives alongside the attention pools (disjoint
                    # SBUF) so the gather DMAs can land while attention
                    # is still running
                    tc.tile_pool(name="ao" + sfx, bufs=1) as ao_pool,
                ):
                    consts = (maskT, ones_col)
                    ao_sb = ao_pool.tile([P, N, S], BF16)

                    def gather_ao(sch):
                        for r in range(4):
                            for l in range(NH_LOC):
                                if sch < 3:
                                    src = ag_out1[r][sch, l * P:(l + 1) * P, :]
                                else:
                                    src = ag_out2[r][l * P:(l + 1) * P, :]
                                nc.sync.dma_start(
                                    ao_sb[:, r * NH_LOC + l,
                                          sch * 512:(sch + 1) * 512], src)

                    with (
                        tc.tile_pool(name="lt_psum" + sfx, bufs=3,
                                     space="PSUM") as lt_psum,
                        tc.tile_pool(name="av_psum" + sfx, bufs=3,
                                     space="PSUM") as av_psum,
                        tc.tile_pool(name="sm_psum" + sfx, bufs=2,
                                     space="PSUM") as sm_psum,
                    ):
                        pools = (elt_pool, qsum_pool, rc_pool, bc_pool,
                                 at_pool, lt_psum, av_psum, sm_psum)
                        pending = []
                        for sc in range(4):
                            _attn_sc(nc, tc, pools, sc, range(NH_LOC), qT_sc,
                                     kT_sc, v_sc, consts, ag_in, pending)
                            if sc == 2:
                                # flush before AG1 so cols 0:1536 are final
                                while pending:
                                    _flush_one(nc, pools, ag_in, pending)
                                if with_cc:
                                    nc.gpsimd.collective_compute(
                                        "AllGather", mybir.AluOpType.bypass,
                                        replica_groups=REPLICA_GROUPS,
                                        ins=[ag_in.ap()[0:3].opt()],
                                        outs=[ag_out1.ap().opt()])
                                for sch in range(3):
                                    gather_ao(sch)
                        while pending:
                            _flush_one(nc, pools, ag_in, pending)
                        if with_cc:
                            nc.gpsimd.collective_compute(
                                "AllGather", mybir.AluOpType.bypass,
                                replica_groups=REPLICA_GROUPS,
                                ins=[ag_in.ap()[3].opt()],
                                outs=[ag_out2.ap().opt()])
                        gather_ao(3)

                    # -------- phase 3: output projection --------
                    with (
                        tc.tile_pool(name="osb" + sfx, bufs=3) as o_pool,
                        tc.tile_pool(name="o_psum" + sfx, bufs=4,
                                     space="PSUM") as op_psum,
                    ):
                        for st in range(ST):
                            ssl = slice(st * P, (st + 1) * P)
                            pso = op_psum.tile([P, ECHUNK], F32, tag="pso")
                            for n in range(N):
                                nc.tensor.matmul(
                                    pso, ao_sb[:, n, ssl], wo_sb[:, n, :],
                                    start=(n == 0), stop=(n == N - 1))
                            osb = o_pool.tile([P, ECHUNK], F32, tag="osb")
                            nc.scalar.copy(out=osb[:], in_=pso[:])
                            nc.scalar.dma_start(out.ap()[ssl, :], osb[:])


def _flush_one(nc, pools, ag_in, pending):
    (elt_pool, qsum_pool, rc_pool, bc_pool, at_pool,
     lt_psum, av_psum, sm_psum) = pools
    n_, sc_, avp_, smp_ = pending.pop(0)
    rc = rc_pool.tile([1, 512], F32, tag="rc")
    nc.vector.reciprocal(rc[:], smp_[:])
    bcs = bc_pool.tile([P, 512], F32, tag="bcs")
    nc.gpsimd.partition_broadcast(bcs[:], rc[:])
    at = at_pool.tile([P, 512], BF16, tag="at")
    nc.vector.tensor_mul(out=at[:], in0=avp_[:], in1=bcs[:])
    nc.sync.dma_start(
        ag_in.ap()[sc_, n_ * P:(n_ + 1) * P, :], at[:])


_NC_CACHE = None


def _get_nc():
    global _NC_CACHE
    if _NC_CACHE is None:
        _NC_CACHE = _build()
    return _NC_CACHE


def kernel(x_q, x_kv, WQ, WK, WV, WO):
    global LAST_RESULTS
    bf = ml_dtypes.bfloat16
    cosT, sinT = _rope_tables()
    wo_flat = WO.reshape(N * H, E)

    in_maps = []
    xT_cache = {}
    for c in range(NCORES):
        b, hg = c // 4, c % 4
        hsl = slice(hg * NH_LOC, (hg + 1) * NH_LOC)
        esl = slice(hg * ECHUNK, (hg + 1) * ECHUNK)
        if b not in xT_cache:
            xT_cache[b] = (
                np.ascontiguousarray(x_q[b].T.astype(bf)),
                np.ascontiguousarray(x_kv[b].T.astype(bf)),
            )
        xqTb, xkTb = xT_cache[b]
        in_maps.append({
            "xqT": xqTb,
            "xkT": xkTb,
            "wq": np.ascontiguousarray(WQ[:, hsl, :].reshape(E, NH_LOC * H).astype(bf)),
            "wk": np.ascontiguousarray(WK[:, hsl, :].reshape(E, NH_LOC * H).astype(bf)),
            "wv": np.ascontiguousarray(WV[:, hsl, :].reshape(E, NH_LOC * H).astype(bf)),
            "wo": np.ascontiguousarray(wo_flat[:, esl].astype(bf)),
            "cosT": cosT,
            "sinT": sinT,
        })

    nc = _get_nc()
    res = run_bass_kernel_spmd(nc, in_maps, core_ids=list(range(NCORES)),
                               trace=TRACE)
    LAST_RESULTS = res

    out = np.empty((B, S, E), dtype=np.float32)
    for c in range(NCORES):
        b, j = c // 4, c % 4
        out[b, :, j * ECHUNK:(j + 1) * ECHUNK] = res.results[c]["out"]
    return out


# revision 19
# speedup vs baseline: 1.1268x; 1.1268x over previous
"""Distributed Trainium2 kernel for causal multi-head attention with RoPE.

Problem: B=2, S=2048, E=2048, N=16 heads, H=128 head-dim.
Sharding: 8 cores = 2 (batch, data axis) x 4 (head groups, model axis).
Each core:
  phase 1: Q/K/V projections for its 4 heads (bf16 matmuls, f32 accum),
           RoPE applied to q^T/k^T in [H, S] layout.  Q runs first over
           all four 512-col s-chunks (sc pairs share one W stationary
           load), then K+V per sc pair, so q^T frees early for the next
           loop iteration and LDWEIGHTS count is halved for Q/K.
  phase 2: causal attention, transposed-score formulation, sc-major:
           for each s-chunk, all 4 heads.  Scores L^T[t,s] (lhsT = k^T
           block, rhs = q^T chunk), exp PSUM->SBUF bf16.  Softmax
           denominators: full 512-wide t-blocks are quad-reduced on the
           DVE (bf16 tree adds) so the ones-column matmul streams 1/4
           of the columns; diagonal blocks feed the matmul directly.
           Normalization (reciprocal -> gpsimd partition_broadcast ->
           DVE multiply) is deferred by one head so the PE never waits
           on it.
  phase 3: two AllGathers per 4-core group split by s-columns (0:1536
           after sc 0-2 finish, 1536:2048 at the end); a^T for all 16
           heads is re-read s-chunk-major so the output projection can
           start on the first s-tile as soon as its chunk lands.  Each
           core computes o for a different 512-wide E-slice.
Host side re-assembles the 8 [2048, 512] E-chunks into [2, 2048, 2048].

DMA queues: phase-1/3 loads on sync (SP), output writes on the scalar
(Activation) queue, a^T stores + collectives on gpsimd (Pool) — so the
sync queue reaches the next iteration's weight loads while the current
iteration is still in attention.
"""

import contextlib

import numpy as np
import ml_dtypes

import concourse.mybir as mybir
import concourse.tile as tile
from concourse import bacc
from concourse.bass_utils import run_bass_kernel_spmd

B, S, E, N, H = 2, 2048, 2048, 16, 128
P = 128
NCORES = 8
NH_LOC = N // 4          # 4 heads per core
ECHUNK = E // 4          # 512 output-embedding columns per core
EC = E // P              # 16 contraction chunks
ST = S // P              # 16 seq tiles of 128
F32 = mybir.dt.float32
BF16 = mybir.dt.bfloat16

REPLICA_GROUPS = [[0, 1, 2, 3], [4, 5, 6, 7]]

TRACE = False
LAST_RESULTS = None
BC_VIA_GPSIMD = True     # reciprocal broadcast on Pool instead of PE matmul


def _rope_tables():
    """cos^T / sin^T tables [H, S] f32, sign-folded and scaled by 128**-0.25.

    Matches reference.sine_table computed in f32 (angles formed with f32
    arithmetic, sin/cos evaluated in f64 of the f32 angle).
    """
    fraction = np.arange(0, H, 2, dtype=np.float32) / np.float32(H)
    timescale = np.float32(10000.0) ** fraction
    inv = (np.float32(1.0) / timescale).astype(np.float32)
    ang = (np.arange(S, dtype=np.float32)[:, None] * inv[None, :]).astype(np.float32)
    ang = np.concatenate([ang, ang], axis=1)        # [S, H]
    sin = np.sin(ang.astype(np.float64))
    cos = np.cos(ang.astype(np.float64))
    scale = float(H) ** -0.25
    cosT = (cos.T * scale).astype(np.float32)        # [H, S]
    sinT = (sin.T * scale).astype(np.float32)
    sinT[:H // 2] *= -1.0                            # sign of rotate_half folded in
    return np.ascontiguousarray(cosT), np.ascontiguousarray(sinT)


def _phase1(nc, tc, qT_sc, kT_sc, v_sc, cos_sb, sin_sb, cos_srcs,
            xqT_r, xkT_r, wq_r, wk_r, wv_r, sfx=""):
    """QKV proj + RoPE -> per-s-chunk qT/kT [h, n, 512] and v [t, tt, nh].

    Pass 1: Q for all four s-chunks (paired, shared W stationary).
    Pass 2: K + V per s-chunk pair (sharing the xk strips).
    """
    with (
        tc.tile_pool(name="wqkv" + sfx, bufs=1) as w_pool,
        tc.tile_pool(name="xin" + sfx, bufs=2) as x_pool,
        tc.tile_pool(name="rope_tmp" + sfx, bufs=3) as tmp_pool,
        tc.tile_pool(name="ph1_psum" + sfx, bufs=2, space="PSUM") as pp1,
    ):
        wq_sb = w_pool.tile([P, EC, NH_LOC * H], BF16)
        nc.sync.dma_start(wq_sb[:, 0:4, :], wq_r[:, 0:4, :])

        def load_strips(xT_r, scs):
            # xq and xk share the tag space: by the time the K+V pass
            # needs strips, the Q pass has consumed the same-tag buffer.
            st = {}
            for si, sc in enumerate(scs):
                cols = slice(sc * 512, (sc + 1) * 512)
                for j in range(4):
                    t = x_pool.tile([P, 4, 512], BF16, tag=f"x{si}{j}")
                    nc.sync.dma_start(t[:], xT_r[:, 4 * j:4 * j + 4, cols])
                    st[(sc, j)] = t
            return st

        xq_st = load_strips(xqT_r, (0, 1))
        # deferred loads, queued behind what pass 1 needs first
        nc.sync.dma_start(cos_sb[:], cos_srcs[0])
        nc.sync.dma_start(sin_sb[:], cos_srcs[1])
        for j in range(1, 4):
            nc.sync.dma_start(wq_sb[:, 4 * j:4 * j + 4, :],
                              wq_r[:, 4 * j:4 * j + 4, :])
        wk_sb = w_pool.tile([P, EC, NH_LOC * H], BF16)
        nc.sync.dma_start(wk_sb[:], wk_r)
        wv_sb = w_pool.tile([P, EC, NH_LOC * H], BF16)

        def rope(ps, dst, cols):
            """dst = ps*cos + shift128(ps)*sin  (bf16 out), cols into S."""
            w = cols.stop - cols.start
            t_sin = tmp_pool.tile([P, 512], F32, tag="t_sin")
            nc.vector.tensor_tensor(
                out=t_sin[0:64, :w], in0=ps[64:128, :w],
                in1=sin_sb[0:64, cols], op=mybir.AluOpType.mult)
            nc.vector.tensor_tensor(
                out=t_sin[64:128, :w], in0=ps[0:64, :w],
                in1=sin_sb[64:128, cols], op=mybir.AluOpType.mult)
            t_cos = tmp_pool.tile([P, 512], F32, tag="t_cos")
            nc.vector.tensor_tensor(
                out=t_cos[:, :w], in0=ps[:, :w],
                in1=cos_sb[:, cols], op=mybir.AluOpType.mult)
            nc.vector.tensor_add(out=dst, in0=t_cos[:, :w], in1=t_sin[:, :w])

        # ---------------- pass 1: Q over sc pairs ----------------
        for half in range(2):
            scs = (2 * half, 2 * half + 1)
            if half == 1:
                xq_st.update(load_strips(xqT_r, scs))
            for n in range(NH_LOC):
                hs = slice(n * H, (n + 1) * H)
                ps = [pp1.tile([P, 512], F32, tag=f"pp{i}", name=f"psq{i}")
                      for i in range(2)]
                for ec in range(EC):
                    for i, sc in enumerate(scs):
                        nc.tensor.matmul(
                            ps[i], wq_sb[:, ec, hs],
                            xq_st[(sc, ec // 4)][:, ec % 4, :],
                            start=(ec == 0), stop=(ec == EC - 1),
                            skip_group_check=True)
                for i, sc in enumerate(scs):
                    rope(ps[i], qT_sc[sc][:, n, :],
                         slice(sc * 512, (sc + 1) * 512))

        # ---------------- pass 2: K + V over sc pairs ----------------
        xk_st = load_strips(xkT_r, (0, 1))
        nc.sync.dma_start(wv_sb[:], wv_r)
        for half in range(2):
            scs = (2 * half, 2 * half + 1)
            if half == 1:
                xk_st.update(load_strips(xkT_r, scs))
            for n in range(NH_LOC):
                hs = slice(n * H, (n + 1) * H)
                ps = [pp1.tile([P, 512], F32, tag=f"pp{i}", name=f"psk{i}")
                      for i in range(2)]
                for ec in range(EC):
                    for i, sc in enumerate(scs):
                        nc.tensor.matmul(
                            ps[i], wk_sb[:, ec, hs],
                            xk_st[(sc, ec // 4)][:, ec % 4, :],
                            start=(ec == 0), stop=(ec == EC - 1),
                            skip_group_check=True)
                for i, sc in enumerate(scs):
                    rope(ps[i], kT_sc[sc][:, n, :],
                         slice(sc * 512, (sc + 1) * 512))
            for sc in scs:
                for tt in range(4):                 # v for 4 t-tiles
                    tsl = slice(tt * P, (tt + 1) * P)
                    psv = pp1.tile([P, 512], F32, tag="psv")
                    for ec in range(EC):
                        nc.tensor.matmul(
                            psv, xk_st[(sc, ec // 4)][:, ec % 4, tsl],
                            wv_sb[:, ec, :],
                            start=(ec == 0), stop=(ec == EC - 1))
                    nc.scalar.copy(out=v_sc[sc][:, tt, :], in_=psv[:])


def _attn_sc(nc, tc, pools, sc, heads, qT_sc, kT_sc, v_sc, consts, ag_in,
             pending):
    """Causal attention chains for s-chunk sc, given heads.

    Normalization of each (sc, head) chain is deferred until after the
    next chain's matmuls are issued (pending holds one entry).
    """
    (elt_pool, qsum_pool, rc_pool, bc_pool, at_pool,
     lt_psum, av_psum, sm_psum) = pools
    maskT, ones_col = consts

    for n in heads:
        hs = slice(n * H, (n + 1) * H)
        avp = av_psum.tile([P, 512], F32, tag="av")
        smp = sm_psum.tile([1, 512], F32, tag="sm")

        # full t-blocks in quads: pv accumulates per block; the block
        # sums are quad-reduced on DVE (bf16 tree) into running ts.
        for q in range(sc):
            elts = []
            for j in range(4):
                tb = 4 * q + j
                kblk = kT_sc[q][:, n, j * P:(j + 1) * P]
                lt = lt_psum.tile([P, 512], F32, tag="lt")
                nc.tensor.matmul(lt[:], kblk, qT_sc[sc][:, n, :],
                                 start=True, stop=True)
                elt = elt_pool.tile([P, 512], BF16, tag=f"elt{j}")
                nc.scalar.activation(
                    out=elt[:], in_=lt[:],
                    func=mybir.ActivationFunctionType.Exp)
                nc.tensor.matmul(
                    avp[:], v_sc[q][:, j, hs], elt[:],
                    start=(tb == 0), stop=False, skip_group_check=True)
                elts.append(elt)
            s01 = qsum_pool.tile([P, 512], BF16, tag="s01")
            nc.vector.tensor_add(out=s01[:], in0=elts[0][:], in1=elts[1][:])
            s23 = qsum_pool.tile([P, 512], BF16, tag="s23")
            nc.vector.tensor_add(out=s23[:], in0=elts[2][:], in1=elts[3][:])
            sq = qsum_pool.tile([P, 512], BF16, tag="sq")
            nc.vector.tensor_add(out=sq[:], in0=s01[:], in1=s23[:])
            nc.tensor.matmul(
                smp[:], ones_col[:], sq[:],
                start=(q == 0), stop=False, skip_group_check=True)

        # diagonal 4 blocks (w = 512, 384, 256, 128): their column sums
        # are accumulated on the DVE into ds so smp gets a single
        # 512-column stream per chain
        ds = qsum_pool.tile([P, 512], BF16, tag="ds")
        for j in range(4):
            tb = 4 * sc + j
            w = 512 - j * P
            col0 = 512 - w
            kblk = kT_sc[sc][:, n, j * P:(j + 1) * P]
            lt = lt_psum.tile([P, 512], F32, tag="lt")
            nc.tensor.matmul(
                lt[:, :w], kblk, qT_sc[sc][:, n, col0:512],
                start=True, stop=True)
            elt = elt_pool.tile([P, 512], BF16, tag=f"elt{j}")
            nc.scalar.activation(
                out=elt[:, :w], in_=lt[:, :w],
                func=mybir.ActivationFunctionType.Exp)
            nc.vector.tensor_mul(
                out=elt[:, 0:P], in0=elt[:, 0:P], in1=maskT[:])
            nc.tensor.matmul(
                avp[:, col0:512], v_sc[sc][:, j, hs], elt[:, :w],
                start=(tb == 0), stop=(j == 3), skip_group_check=True)
            if j == 0:
                nc.vector.tensor_copy(out=ds[:], in_=elt[:])
            else:
                nc.vector.tensor_add(
                    out=ds[:, col0:512], in0=ds[:, col0:512],
                    in1=elt[:, :w])
        nc.tensor.matmul(
            smp[:], ones_col[:], ds[:],
            start=(sc == 0), stop=True, skip_group_check=True)

        if pending:
            _flush_one(nc, pools, ag_in, pending)
        pending.append((n, sc, avp, smp))


def _build(reps=1, with_cc=True, loop_trips=0):
    nc = bacc.Bacc("TRN2", target_bir_lowering=False, debug=False,
                   num_devices=NCORES)

    xqT = nc.dram_tensor("xqT", [E, S], BF16, kind="ExternalInput")
    xkT = nc.dram_tensor("xkT", [E, S], BF16, kind="ExternalInput")
    wq = nc.dram_tensor("wq", [E, NH_LOC * H], BF16, kind="ExternalInput")
    wk = nc.dram_tensor("wk", [E, NH_LOC * H], BF16, kind="ExternalInput")
    wv = nc.dram_tensor("wv", [E, NH_LOC * H], BF16, kind="ExternalInput")
    wo = nc.dram_tensor("wo", [N * H, ECHUNK], BF16, kind="ExternalInput")
    cosT = nc.dram_tensor("cosT", [H, S], F32, kind="ExternalInput")
    sinT = nc.dram_tensor("sinT", [H, S], F32, kind="ExternalInput")
    out = nc.dram_tensor("out", [S, ECHUNK], F32, kind="ExternalOutput")

    # s-chunk-major so each AllGather input slice is contiguous
    ag_in = nc.dram_tensor("ag_in", [4, NH_LOC * H, 512], BF16)
    ag_out1 = nc.dram_tensor("ag_out1", [4, 3, NH_LOC * H, 512], BF16)
    ag_out2 = nc.dram_tensor("ag_out2", [4, NH_LOC * H, 512], BF16)

    xqT_r = xqT.ap().rearrange("(eo p) s -> p eo s", p=P)    # [128, 16, 2048]
    xkT_r = xkT.ap().rearrange("(eo p) s -> p eo s", p=P)
    wq_r = wq.ap().rearrange("(eo p) m -> p eo m", p=P)      # [128, 16, 512]
    wk_r = wk.ap().rearrange("(eo p) m -> p eo m", p=P)
    wv_r = wv.ap().rearrange("(eo p) m -> p eo m", p=P)
    wo_r = wo.ap().rearrange("(n p) e -> p n e", p=P)        # [128, 16, 512]

    with tile.TileContext(nc) as tc:
        with tc.tile_pool(name="const", bufs=1) as const_pool:
            # multiplicative causal mask for L^T diagonal blocks:
            # keep where s - t >= 0
            maskT = const_pool.tile([P, P], BF16)
            nc.gpsimd.memset(maskT, 1.0)
            nc.gpsimd.affine_select(
                out=maskT, in_=maskT,
                compare_op=mybir.AluOpType.is_ge, fill=0.0,
                base=0, pattern=[[1, P]], channel_multiplier=-1)
            ones_col = const_pool.tile([P, 1], BF16)
            nc.gpsimd.memset(ones_col, 1.0)
            cos_sb = const_pool.tile([P, S], F32)
            sin_sb = const_pool.tile([P, S], F32)
            cos_srcs = (cosT.ap(), sinT.ap())

            loop_cm = tc.For_i(0, loop_trips, 1) if loop_trips else \
                contextlib.nullcontext()
            with loop_cm:
                _build_body(nc, tc, reps, with_cc, out, ag_in,
                            ag_out1, ag_out2, cos_sb, sin_sb,
                            cos_srcs, maskT, ones_col,
                            xqT_r, xkT_r, wq_r, wk_r, wv_r, wo_r)

    nc.compile()
    return nc


def _build_body(nc, tc, reps, with_cc, out, ag_in, ag_out1, ag_out2,
                cos_sb, sin_sb, cos_srcs, maskT, ones_col, xqT_r, xkT_r,
                wq_r, wk_r, wv_r, wo_r):
    for rep in range(reps):
        sfx = f"_r{rep}" if reps > 1 else ""
        with tc.tile_pool(name="qkv" + sfx, bufs=1) as qkv_pool:
            qT_sc = [qkv_pool.tile([P, NH_LOC, 512], BF16, name=f"qT{sc}")
                     for sc in range(4)]
            kT_sc = [qkv_pool.tile([P, NH_LOC, 512], BF16, name=f"kT{sc}")
                     for sc in range(4)]
            v_sc = [qkv_pool.tile([P, 4, NH_LOC * H], BF16, name=f"v{sc}")
                    for sc in range(4)]

            _phase1(nc, tc, qT_sc, kT_sc, v_sc, cos_sb, sin_sb, cos_srcs,
                    xqT_r, xkT_r, wq_r, wk_r, wv_r, sfx)

            # prefetch WO for phase 3 (SBUF freed by phase-1 pools)
            with tc.tile_pool(name="wo_pool" + sfx, bufs=1) as wo_pool:
                wo_sb = wo_pool.tile([P, N, ECHUNK], BF16)
                nc.sync.dma_start(wo_sb[:], wo_r)

                with (
                    tc.tile_pool(name="elt" + sfx, bufs=2) as elt_pool,
                    tc.tile_pool(name="qsum" + sfx, bufs=2) as qsum_pool,
                    tc.tile_pool(name="rc" + sfx, bufs=2) as rc_pool,
                    tc.tile_pool(name="bc" + sfx, bufs=2) as bc_pool,
                    tc.tile_pool(name="at" + sfx, bufs=2) as at_pool,
                    # ao lives alongside the attention pools (disjoint
                    # SBUF) so the gather DMAs can land while attention
                    # is still running
                    tc.tile_pool(name="ao" + sfx, bufs=1) as ao_pool,
                ):
                    consts = (maskT, ones_col)
                    ao_sb = ao_pool.tile([P, N, S], BF16)

                    def gather_ao(sch):
                        for r in range(4):
                            for l in range(NH_LOC):
                                if sch < 3:
                                    src = ag_out1[r][sch, l * P:(l + 1) * P, :]
                                else:
                                    src = ag_out2[r][l * P:(l + 1) * P, :]
                                nc.sync.dma_start(
                                    ao_sb[:, r * NH_LOC + l,
                                          sch * 512:(sch + 1) * 512], src)

                    with (
                        tc.tile_pool(name="lt_psum" + sfx, bufs=3,
                                     space="PSUM") as lt_psum,
                        tc.tile_pool(name="av_psum" + sfx, bufs=3,
                                     space="PSUM") as av_psum,
                        tc.tile_pool(name="sm_psum" + sfx, bufs=2,
                                     space="PSUM") as sm_psum,
                    ):
                        pools = (elt_pool, qsum_pool, rc_pool, bc_pool,
                                 at_pool, lt_psum, av_psum, sm_psum)
                        pending = []
                        for sc in range(4):
                            _attn_sc(nc, tc, pools, sc, range(NH_LOC), qT_sc,
                                     kT_sc, v_sc, consts, ag_in, pending)
                            if sc == 2:
                                # flush before AG1 so cols 0:1536 are final
                                while pending:
                                    _flush_one(nc, pools, ag_in, pending)
                                if with_cc:
                                    nc.gpsimd.collective_compute(
                                        "AllGather", mybir.AluOpType.bypass,
                                        replica_groups=REPLICA_GROUPS,
                                        ins=[ag_in.ap()[0:3].opt()],
                                        outs=[ag_out1.ap().opt()])
                                for sch in range(3):
                                    gather_ao(sch)
                        while pending:
                            _flush_one(nc, pools, ag_in, pending)
                        if with_cc:
                            nc.gpsimd.collective_compute(
                                "AllGather", mybir.AluOpType.bypass,
                                replica_groups=REPLICA_GROUPS,
                                ins=[ag_in.ap()[3].opt()],
                                outs=[ag_out2.ap().opt()])
                        gather_ao(3)

                    # -------- phase 3: output projection --------
                    with (
                        tc.tile_pool(name="osb" + sfx, bufs=3) as o_pool,
                        tc.tile_pool(name="o_psum" + sfx, bufs=4,
                                     space="PSUM") as op_psum,
                    ):
                        for st in range(ST):
                            ssl = slice(st * P, (st + 1) * P)
                            pso = op_psum.tile([P, ECHUNK], F32, tag="pso")
                            for n in range(N):
                                nc.tensor.matmul(
                                    pso, ao_sb[:, n, ssl], wo_sb[:, n, :],
                                    start=(n == 0), stop=(n == N - 1))
                            osb = o_pool.tile([P, ECHUNK], F32, tag="osb")
                            nc.scalar.copy(out=osb[:], in_=pso[:])
                            nc.scalar.dma_start(out.ap()[ssl, :], osb[:])


def _flush_one(nc, pools, ag_in, pending):
    (elt_pool, qsum_pool, rc_pool, bc_pool, at_pool,
     lt_psum, av_psum, sm_psum) = pools
    n_, sc_, avp_, smp_ = pending.pop(0)
    rc = rc_pool.tile([1, 512], F32, tag="rc")
    nc.vector.reciprocal(rc[:], smp_[:])
    bcs = bc_pool.tile([P, 512], F32, tag="bcs")
    nc.gpsimd.partition_broadcast(bcs[:], rc[:])
    at = at_pool.tile([P, 512], BF16, tag="at")
    nc.vector.tensor_mul(out=at[:], in0=avp_[:], in1=bcs[:])
    nc.sync.dma_start(
        ag_in.ap()[sc_, n_ * P:(n_ + 1) * P, :], at[:])


_NC_CACHE = None


def _get_nc():
    global _NC_CACHE
    if _NC_CACHE is None:
        _NC_CACHE = _build()
    return _NC_CACHE


def kernel(x_q, x_kv, WQ, WK, WV, WO):
    global LAST_RESULTS
    bf = ml_dtypes.bfloat16
    cosT, sinT = _rope_tables()
    wo_flat = WO.reshape(N * H, E)

    in_maps = []
    xT_cache = {}
    for c in range(NCORES):
        b, hg = c // 4, c % 4
        hsl = slice(hg * NH_LOC, (hg + 1) * NH_LOC)
        esl = slice(hg * ECHUNK, (hg + 1) * ECHUNK)
        if b not in xT_cache:
            xT_cache[b] = (
                np.ascontiguousarray(x_q[b].T.astype(bf)),
                np.ascontiguousarray(x_kv[b].T.astype(bf)),
            )
        xqTb, xkTb = xT_cache[b]
        in_maps.append({
            "xqT": xqTb,
            "xkT": xkTb,
            "wq": np.ascontiguousarray(WQ[:, hsl, :].reshape(E, NH_LOC * H).astype(bf)),
            "wk": np.ascontiguousarray(WK[:, hsl, :].reshape(E, NH_LOC * H).astype(bf)),
            "wv": np.ascontiguousarray(WV[:, hsl, :].reshape(E, NH_LOC * H).astype(bf)),
            "wo": np.ascontiguousarray(wo_flat[:, esl].astype(bf)),
            "cosT": cosT,
            "sinT": sinT,
        })

    nc = _get_nc()
    res = run_bass_kernel_spmd(nc, in_maps, core_ids=list(range(NCORES)),
                               trace=TRACE)
    LAST_RESULTS = res

    out = np.empty((B, S, E), dtype=np.float32)
    for c in range(NCORES):
        b, j = c // 4, c % 4
        out[b, :, j * ECHUNK:(j + 1) * ECHUNK] = res.results[c]["out"]
    return out
